# revision 9
# baseline (speedup 1.0000x reference)
"""Adaptive average pooling 2D on 8 TRN2 NeuronCores.

Input  x: (16, 224, 224, 128) f32, channels_last.
Output:   (16, 7, 7, 128) f32.

Since 224 = 7*32, the adaptive bins are uniform 32x32 windows, so
out[b,i,j,c] = mean of the 32x32 spatial block (i,j) of sample b.

Sharding: data parallel over batch -> 2 samples per core, no comms.

Per-core algorithm (x viewed as rows of (W*C)=28672 contiguous f32):
  - h-reduction via TensorEngine matmul with a block-diagonal weight
    lhsT[K=128, M=4] (value 1/1024 on 32-row blocks): psum[m, :] =
    sum over the m-th 32-row block. fp32r dtype -> 1 cycle/column.
  - w-reduction: each DMA tile is one (h-chunk, j) window [128, 32*128];
    8 matmuls over its 512-wide column chunks accumulate in one PSUM
    bank, leaving a 4-way strided sum done on the VectorEngine.
  - results collect in one SBUF tile [14, 896] = (b,i) x (j,c), single
    DMA out.
"""

import numpy as np

B, H, W, C = 16, 224, 224, 128
NCORES = 8
BPC = B // NCORES  # samples per core
OUT_H = OUT_W = 7
BLK = 32  # pooling window edge (224/7)
ROWC = W * C  # 28672 contiguous f32 per (b, h) row
# h-chunks per sample: rows [0,128) = 4 blocks, rows [128,224) = 3 blocks
H_CHUNKS = ((0, 128, 4), (128, 96, 3))
INV_AREA = 1.0 / float(BLK * BLK)

_NC = None


def _build_nc():
    import concourse.bacc as bacc
    import concourse.mybir as mybir
    import concourse.tile as tile

    f32 = mybir.dt.float32
    f32r = mybir.dt.float32r

    nc = bacc.Bacc("TRN2", target_bir_lowering=False, debug=False,
                   enable_asserts=False)
    x_ext = nc.dram_tensor("x", [BPC * H, ROWC], f32r, kind="ExternalInput")
    w_ext = nc.dram_tensor("w", [128, 4], f32r, kind="ExternalInput")
    out_ext = nc.dram_tensor("out", [BPC * OUT_H, OUT_W * C], f32,
                             kind="ExternalOutput")

    with tile.TileContext(nc) as tc:
        with (
            tc.tile_pool(name="const", bufs=1) as cpool,
            tc.tile_pool(name="inp", bufs=3) as ipool,
            tc.tile_pool(name="acc", bufs=4, space="PSUM") as ppool,
            tc.tile_pool(name="res", bufs=1) as opool,
        ):
            wtile = cpool.tile([128, 4], f32r)
            nc.sync.dma_start(out=wtile[:, :], in_=w_ext[:, :])

            # otile partitions = m (h-block within chunk), free = (hc,b,j,c)
            otile = opool.tile([4, 2 * BPC * OUT_W * C], f32)

            for b in range(BPC):
                for hc, (r0, K, M) in enumerate(H_CHUNKS):
                    row0 = b * H + r0
                    for j in range(OUT_W):
                        off = ((hc * BPC + b) * OUT_W + j) * C
                        t = ipool.tile([128, BLK * C], f32r)
                        nc.sync.dma_start(
                            out=t[:K, :],
                            in_=x_ext[row0:row0 + K,
                                      j * BLK * C:(j + 1) * BLK * C],
                        )
                        p = ppool.tile([4, 512], f32)
                        for k in range(8):
                            nc.tensor.matmul(
                                p[:M, :],
                                wtile[:K, :M],
                                t[:K, k * 512:(k + 1) * 512],
                                start=(k == 0),
                                stop=(k == 7),
                            )
                        nc.vector.reduce_sum(
                            otile[:M, off:off + C],
                            p[:M, :].rearrange("p (u c) -> p c u", u=4),
                            axis=mybir.AxisListType.X,
                        )

            # out DRAM is (b, i, j, c) flattened to [BPC*7, 7*C]; SBUF free
            # within one hc slice is (b, j, c). i = hc*4 + m.
            dview = out_ext[:, :].rearrange(
                "(b i) (j c) -> i b j c", b=BPC, j=OUT_W)
            for hc, (_, _, M) in enumerate(H_CHUNKS):
                sl = otile[:M, hc * BPC * OUT_W * C:(hc + 1) * BPC * OUT_W * C]
                nc.sync.dma_start(
                    out=dview[hc * 4:hc * 4 + M],
                    in_=sl.rearrange("m (b j c) -> m b j c", b=BPC, j=OUT_W),
                )

    nc.compile()
    return nc


def _get_nc():
    global _NC
    if _NC is None:
        _NC = _build_nc()
    return _NC


def _weight() -> np.ndarray:
    w = np.zeros((128, 4), dtype=np.float32)
    for m in range(4):
        w[32 * m:32 * m + 32, m] = INV_AREA
    return w


def kernel(x: np.ndarray) -> np.ndarray:
    from concourse.bass_utils import run_bass_kernel_spmd

    nc = _get_nc()
    x = np.ascontiguousarray(np.asarray(x, dtype=np.float32))
    assert x.shape == (B, H, W, C)
    w = _weight()
    in_maps = [
        {"x": x[BPC * c:BPC * (c + 1)].reshape(BPC * H, ROWC), "w": w}
        for c in range(NCORES)
    ]
    res = run_bass_kernel_spmd(nc, in_maps, core_ids=list(range(NCORES)))
    outs = [r["out"].reshape(BPC, OUT_H, OUT_W, C) for r in res.results]
    return np.concatenate(outs, axis=0)
